# revision 3
# baseline (speedup 1.0000x reference)
"""GRU-with-skip Trainium2 kernel (v2: column-tiled recurrence).

Strategy (data-parallel over batch, 8 cores, B_local=16 per core):
  Phase 1: input projections rx/(-zx)/nx/skip = x @ W*.T + b as 128-row
           tiles (PE-transposed x as lhsT, fp32r). Staged to DRAM in
           recurrence-friendly layouts:
             rzx_st [T, 64, 512]  rows 16g+b = batch b, h-chunk g;
                                  free = [rx chunk | -zx chunk]
             nx_st  [T, 128, 256] rows 32g+b (padded), free = nx chunk
             sk_st  [16, T, 1024] bf16 (dense, per-batch)
  Phase 2: recurrence. Gate pre-activations are computed with 4-way
           PE column tiling: stationary = hT ko-tile [128, 16] shared
           by 4 col-groups (tile_position (0, 32g)); each group streams
           its own 256-wide slice of the (fused, z-negated) hidden
           weights. PSUM layout packs 4 h-chunks at partition bases
           0/32/64/96, so gate math runs as [128, 256] DVE/ACT ops.
           rx/zx/bhn adds are folded into PSUM with a scatter-matmul
           (S: [64->128] row-scatter identity) that also initializes
           the accumulation (start=True over the full bank).
           h_new is re-transposed with 2 full-array PE transposes.
  Phase 3: skip-add + LayerNorm + output projection (gamma/beta folded
           into Wout/bout on the host).

All matmuls in float32r; gate math fp32; staging rzx/nx fp32, hs/skip bf16.
"""

import sys

for _p in ("/opt/trn_rl_repo", "/root/.axon_site/_ro/trn_rl_repo"):
    if _p not in sys.path:
        sys.path.insert(0, _p)

import numpy as np

import concourse.bass as bass
import concourse.tile as tile
from concourse import bacc, mybir
from concourse.bass_utils import run_bass_kernel_spmd

F32 = mybir.dt.float32
F32R = mybir.dt.float32r
BF16 = mybir.dt.bfloat16
AF = mybir.ActivationFunctionType
ALU = mybir.AluOpType

P = 128
B, T, I, H, O = 128, 1024, 512, 1024, 512
NCORES = 8
BC = B // NCORES  # 16 batch rows per core
NG = 4  # psum column groups
HC = H // NG  # 256: h-chunk per group
LN_EPS = 1e-5


def build_nc(t_steps: int = T):
    nc = bacc.Bacc(None, target_bir_lowering=False)

    # ---- I/O ----
    x_in = nc.dram_tensor("x", [BC, t_steps, I], F32, kind="ExternalInput")
    wiT = nc.dram_tensor("wiT", [I, 4 * H], F32R, kind="ExternalInput")
    bias_i = nc.dram_tensor("bias_i", [P, 4 * H], F32R, kind="ExternalInput")
    whT = nc.dram_tensor("whT", [H, 3 * H], F32R, kind="ExternalInput")
    bn_d = nc.dram_tensor("bn_d", [4 * BC, HC], F32R, kind="ExternalInput")
    scat = nc.dram_tensor("scat", [4 * BC, P], F32R, kind="ExternalInput")
    woT = nc.dram_tensor("woT", [H, O], F32R, kind="ExternalInput")
    bias_o = nc.dram_tensor("bias_o", [P, O], F32R, kind="ExternalInput")
    ones128 = nc.dram_tensor("ones128", [P, P], F32R, kind="ExternalInput")
    ident = nc.dram_tensor("ident", [P, P], F32, kind="ExternalInput")
    out = nc.dram_tensor("out", [BC, t_steps, O], F32, kind="ExternalOutput")

    n_rt = (BC * t_steps) // P  # number of 128-row tiles
    tpb = t_steps // P  # row-tiles ("time blocks") per batch row

    with tile.TileContext(nc) as tc:
        with (
            tc.tile_pool(name="dram", bufs=1, space="DRAM") as dram,
            tc.tile_pool(name="const", bufs=1) as const,
        ):
            # DRAM staging
            rzx_st = dram.tile([t_steps, 4 * BC, 2 * HC], F32R)
            nx_st = dram.tile([t_steps, P, HC], F32)
            hs_st = dram.tile([t_steps, P, HC], BF16)
            sk_st = dram.tile([BC, t_steps, H], BF16)

            ident_sb = const.tile([P, P], F32)
            nc.sync.dma_start(ident_sb, ident[:])

            # ================= Phase 1: input projections =================
            with (
                tc.tile_pool(name="p1w", bufs=1) as p1w,
                tc.tile_pool(name="p1s", bufs=3) as p1s,
                tc.tile_pool(name="p1e", bufs=3) as p1e,
                tc.tile_pool(name="psA", bufs=2, space="PSUM") as psA,
                tc.tile_pool(name="psB", bufs=4, space="PSUM") as psB,
            ):
                wiT_sb = p1w.tile([P, I // P, 4 * H], F32R)
                nc.sync.dma_start(
                    wiT_sb, wiT[:].rearrange("(ko p) m -> p ko m", p=P)
                )
                bias_i_sb = p1w.tile([P, 4 * H], F32R)
                nc.sync.dma_start(bias_i_sb, bias_i[:])
                ones128_sb = p1w.tile([P, P], F32R)
                nc.sync.dma_start(ones128_sb, ones128[:])

                for rt in range(n_rt):
                    b = rt // tpb
                    t0 = (rt % tpb) * P
                    xt = p1s.tile([P, I], F32)
                    nc.sync.dma_start(xt, x_in[b, t0 : t0 + P, :])
                    px = psA.tile([P, I // P, P], F32)
                    for j in range(I // P):
                        nc.tensor.transpose(
                            px[:, j], xt[:, j * P : (j + 1) * P], ident_sb
                        )
                    xT = p1s.tile([P, I // P, P], F32R, tag="xT")
                    nc.vector.tensor_copy(xT, px)
                    for m in range(4):
                        if m < 2:
                            ev = p1e.tile([P, NG, HC], F32R, tag=f"ev{m}")
                        elif m == 2:
                            ev = p1e.tile([P, NG, HC], F32, tag="ev2")
                        else:
                            ev = p1e.tile([P, H], BF16, tag="ev3")
                        for c in range(2):
                            col = m * H + c * 512
                            pm = psB.tile([P, 512], F32)
                            for ko in range(I // P):
                                nc.tensor.matmul(
                                    pm,
                                    xT[:, ko],
                                    wiT_sb[:, ko, col : col + 512],
                                    start=(ko == 0),
                                    stop=False,
                                )
                            nc.tensor.matmul(
                                pm,
                                ones128_sb,
                                bias_i_sb[:, col : col + 512],
                                start=False,
                                stop=True,
                            )
                            if m < 3:
                                dstv = ev[:, 2 * c : 2 * c + 2, :]
                            else:
                                dstv = ev[:, c * 512 : (c + 1) * 512]
                            if c == 0:
                                nc.vector.tensor_copy(dstv, pm)
                            else:
                                nc.scalar.copy(dstv, pm)
                        if m == 0:  # r -> rzx_st free 0:HC
                            nc.sync.dma_start(
                                rzx_st[t0 : t0 + P, b::BC, 0:HC], ev
                            )
                        elif m == 1:  # -z -> rzx_st free HC:2HC
                            nc.sync.dma_start(
                                rzx_st[t0 : t0 + P, b::BC, HC : 2 * HC], ev
                            )
                        elif m == 2:  # n -> nx_st (padded rows 32g+b)
                            nc.sync.dma_start(
                                nx_st[t0 : t0 + P, b :: 2 * BC, :], ev
                            )
                        else:  # skip (bf16, dense)
                            nc.sync.dma_start(sk_st[b, t0 : t0 + P, :], ev)

            # ================= Phase 2: recurrence =================
            with (
                tc.tile_pool(name="p2w", bufs=1) as p2w,
                tc.tile_pool(name="p2s", bufs=4) as p2s,
                tc.tile_pool(name="p2t", bufs=2) as p2t,
                tc.tile_pool(name="pgr", bufs=1, space="PSUM") as pgr,
                tc.tile_pool(name="pgz", bufs=1, space="PSUM") as pgz,
                tc.tile_pool(name="pgn", bufs=1, space="PSUM") as pgn,
                tc.tile_pool(name="ptr", bufs=2, space="PSUM") as ptr_pool,
            ):
                whT_sb = p2w.tile([P, H // P, 3 * H], F32R)
                nc.sync.dma_start(
                    whT_sb, whT[:].rearrange("(ko p) m -> p ko m", p=P)
                )
                scat_sb = p2w.tile([4 * BC, P], F32R)
                nc.sync.dma_start(scat_sb, scat[:])
                bn_sb = p2w.tile([4 * BC, HC], F32R)
                nc.sync.dma_start(bn_sb, bn_d[:])

                # initial state h=0 (padded layout [128, 256])
                h_prev = p2t.tile([P, HC], F32, tag="h")
                nc.vector.memset(h_prev, 0.0)
                hT_f32 = p2t.tile([P, 2, P], F32, tag="hTf")
                nc.vector.memset(hT_f32, 0.0)
                hT_prev = p2t.tile([P, 2, P], F32R, tag="hT")
                nc.vector.tensor_copy(hT_prev, hT_f32)

                def lhs(j):
                    return hT_prev[:, j % 2, 32 * (j // 2) : 32 * (j // 2) + BC]

                for t in range(t_steps):
                    rzx_t = p2s.tile([4 * BC, 2 * HC], F32R, tag="rzx")
                    nc.sync.dma_start(rzx_t, rzx_st[t])
                    nx_t = p2s.tile([P, HC], F32, tag="nx")
                    nc.sync.dma_start(nx_t, nx_st[t])

                    ps_r = pgr.tile([P, HC], F32, tag="psr")
                    ps_z = pgz.tile([P, HC], F32, tag="psz")
                    ps_n = pgn.tile([P, HC], F32, tag="psn")
                    # scatter-inits: fold rx / (-zx) / bhn into PSUM
                    nc.tensor.matmul(
                        ps_r, scat_sb, rzx_t[:, 0:HC], start=True, stop=False
                    )
                    nc.tensor.matmul(
                        ps_z, scat_sb, rzx_t[:, HC : 2 * HC], start=True, stop=False
                    )
                    nc.tensor.matmul(ps_n, scat_sb, bn_sb, start=True, stop=False)

                    # gate matmuls: r (gate 0), then n (gate 2), then z (gate 1)
                    for gate, ps in ((0, ps_r), (2, ps_n), (1, ps_z)):
                        for j in range(H // P):
                            lh = lhs(j)
                            for g in range(NG):
                                col = gate * H + HC * g
                                nc.tensor.matmul(
                                    ps[32 * g : 32 * g + BC, :],
                                    lh,
                                    whT_sb[:, j, col : col + HC],
                                    start=False,
                                    stop=(j == H // P - 1 and g == NG - 1),
                                    tile_position=(0, 32 * g),
                                )
                        if gate == 0:
                            r_sb = p2t.tile([P, HC], F32, tag="r")
                            nc.scalar.activation(r_sb, ps_r, AF.Sigmoid)
                        elif gate == 2:
                            t1 = p2t.tile([P, HC], F32, tag="t1")
                            nc.vector.tensor_mul(t1, r_sb, ps_n)
                            t2 = p2t.tile([P, HC], F32, tag="t2")
                            nc.vector.tensor_add(t2, t1, nx_t)
                            n_sb = p2t.tile([P, HC], F32, tag="n")
                            nc.scalar.activation(n_sb, t2, AF.Tanh)
                            d_sb = p2t.tile([P, HC], F32, tag="d")
                            nc.vector.tensor_sub(d_sb, n_sb, h_prev)

                    zp_sb = p2t.tile([P, HC], F32, tag="zp")
                    nc.scalar.activation(zp_sb, ps_z, AF.Sigmoid)
                    e_sb = p2t.tile([P, HC], F32, tag="e")
                    nc.vector.tensor_mul(e_sb, zp_sb, d_sb)
                    h_new = p2t.tile([P, HC], F32, tag="h")
                    nc.vector.tensor_add(h_new, h_prev, e_sb)

                    # re-transpose h for next step's stationary operand
                    ptr1 = ptr_pool.tile([P, P], F32, tag="ptr1")
                    nc.tensor.transpose(ptr1, h_new[:, 0:P], ident_sb)
                    ptr2 = ptr_pool.tile([P, P], F32, tag="ptr2")
                    nc.tensor.transpose(ptr2, h_new[:, P : 2 * P], ident_sb)
                    hT_new = p2t.tile([P, 2, P], F32R, tag="hT")
                    nc.scalar.copy(hT_new[:, 0, :], ptr1)
                    nc.scalar.copy(hT_new[:, 1, :], ptr2)

                    h16 = p2s.tile([P, HC], BF16, tag="h16")
                    nc.scalar.copy(h16, h_new)
                    nc.sync.dma_start(hs_st[t], h16)

                    h_prev, hT_prev = h_new, hT_new

            # ================= Phase 3: skip + LN + out proj =================
            with (
                tc.tile_pool(name="p3w", bufs=1) as p3w,
                tc.tile_pool(name="p3s", bufs=3) as p3s,
                tc.tile_pool(name="p3t", bufs=2) as p3t,
                tc.tile_pool(name="ps3", bufs=2, space="PSUM") as ps3,
                tc.tile_pool(name="ps4", bufs=2, space="PSUM") as ps4,
            ):
                woT_sb = p3w.tile([P, H // P, O], F32R)
                nc.sync.dma_start(woT_sb, woT[:].rearrange("(ko p) m -> p ko m", p=P))
                bias_o_sb = p3w.tile([P, O], F32R)
                nc.sync.dma_start(bias_o_sb, bias_o[:])
                ones128_sb3 = p3w.tile([P, P], F32R)
                nc.sync.dma_start(ones128_sb3, ones128[:])
                eps_sb = p3w.tile([P, 1], F32)
                nc.vector.memset(eps_sb, LN_EPS)

                for rt in range(n_rt):
                    b = rt // tpb
                    t0 = (rt % tpb) * P
                    hs_t = p3s.tile([P, NG, HC], BF16, tag="hs")
                    nc.sync.dma_start(hs_t, hs_st[t0 : t0 + P, b :: 2 * BC, :])
                    sk_t = p3s.tile([P, H], BF16, tag="sk")
                    nc.sync.dma_start(sk_t, sk_st[b, t0 : t0 + P, :])
                    comb = p3t.tile([P, H], F32, tag="comb")
                    nc.vector.tensor_add(
                        comb, hs_t[:].rearrange("p g c -> p (g c)"), sk_t
                    )

                    st = p3t.tile([P, 2, 6], F32, tag="st")
                    nc.vector.bn_stats(st[:, 0], comb[:, :512])
                    nc.vector.bn_stats(st[:, 1], comb[:, 512:])
                    mv = p3t.tile([P, 2], F32, tag="mv")
                    nc.vector.bn_aggr(mv, st)
                    rstd = p3t.tile([P, 1], F32, tag="rstd")
                    nc.scalar.activation(rstd, mv[:, 1:2], AF.Sqrt, bias=eps_sb)
                    nc.vector.reciprocal(rstd, rstd)
                    normed = p3t.tile([P, H], F32, tag="normed")
                    nc.vector.tensor_scalar(
                        out=normed,
                        in0=comb,
                        scalar1=mv[:, 0:1],
                        scalar2=rstd,
                        op0=ALU.subtract,
                        op1=ALU.mult,
                    )

                    nT = p3t.tile([P, H // P, P], F32R, tag="nT")
                    for j2 in range(2):
                        pn = ps3.tile([P, 4, P], F32, tag="pn")
                        for j in range(4):
                            jj = j2 * 4 + j
                            nc.tensor.transpose(
                                pn[:, j], normed[:, jj * P : (jj + 1) * P], ident_sb
                            )
                        nc.vector.tensor_copy(nT[:, j2 * 4 : j2 * 4 + 4], pn)

                    po = ps4.tile([P, O], F32, tag="po")
                    for ko in range(H // P):
                        nc.tensor.matmul(
                            po, nT[:, ko], woT_sb[:, ko], start=(ko == 0), stop=False
                        )
                    nc.tensor.matmul(
                        po, ones128_sb3, bias_o_sb, start=False, stop=True
                    )
                    o_sb = p3t.tile([P, O], F32, tag="o")
                    nc.scalar.copy(o_sb, po)
                    nc.sync.dma_start(out[b, t0 : t0 + P, :], o_sb)

    nc.finalize()
    return nc


def prep_host_inputs(inputs):
    """Build the shared (weight) input arrays from the full problem inputs."""
    g = {k: np.asarray(v, dtype=np.float32) for k, v in inputs.items()}
    # z-path negated so one sigmoid gives zp = 1 - z directly
    wiT = np.concatenate(
        [g["Wir"].T, -g["Wiz"].T, g["Win"].T, g["Wskip"].T], axis=1
    )  # [I, 4H]
    bias_i = np.zeros((P, 4 * H), np.float32)
    bias_i[0, 0:H] = g["bir"] + g["bhr"]
    bias_i[0, H : 2 * H] = -(g["biz"] + g["bhz"])
    bias_i[0, 2 * H : 3 * H] = g["bin_"]
    bias_i[0, 3 * H :] = g["bskip"]
    whT = np.concatenate([g["Whr"].T, -g["Whz"].T, g["Whn"].T], axis=1)  # [H, 3H]
    # bn_d[16g+b, c] = bhn[256g+c]
    bn_d = np.broadcast_to(
        g["bhn"].reshape(NG, 1, HC), (NG, BC, HC)
    ).reshape(NG * BC, HC).copy()
    # scat[16g+b, 32g+b] = 1
    scat = np.zeros((NG * BC, P), np.float32)
    for gg in range(NG):
        for bb in range(BC):
            scat[BC * gg + bb, 32 * gg + bb] = 1.0
    woT = np.ascontiguousarray((g["Wout"] * g["gamma"][None, :]).T)  # [H, O]
    bias_o = np.zeros((P, O), np.float32)
    bias_o[0] = g["bout"] + g["Wout"] @ g["beta"]
    ones128 = np.zeros((P, P), np.float32)
    ones128[0] = 1.0
    ident = np.eye(P, dtype=np.float32)
    return dict(
        wiT=np.ascontiguousarray(wiT),
        bias_i=bias_i,
        whT=np.ascontiguousarray(whT),
        bn_d=bn_d,
        scat=scat,
        woT=woT,
        bias_o=bias_o,
        ones128=ones128,
        ident=ident,
    )


_NC_CACHE = {}


def run(inputs, t_steps=T, trace=False):
    if t_steps not in _NC_CACHE:
        _NC_CACHE[t_steps] = build_nc(t_steps)
    nc = _NC_CACHE[t_steps]
    shared = prep_host_inputs(inputs)
    x = np.asarray(inputs["x"], dtype=np.float32)[:, :t_steps, :]
    in_maps = [
        {"x": np.ascontiguousarray(x[c * BC : (c + 1) * BC]), **shared}
        for c in range(NCORES)
    ]
    res = run_bass_kernel_spmd(
        nc, in_maps, core_ids=list(range(NCORES)), trace=trace
    )
    outp = np.concatenate([res.results[c]["out"] for c in range(NCORES)], axis=0)
    return outp, res


def kernel(**inputs) -> np.ndarray:
    outp, _ = run(inputs)
    return outp


# revision 6
# speedup vs baseline: 1.4434x; 1.4434x over previous
"""GRU-with-skip Trainium2 kernel (v2: 4-way column-tiled recurrence, bf16).

Strategy (data-parallel over batch, 8 cores, B_local=16 per core):
  Phase 1: input projections rx/(-zx)/nx/skip = x @ W*.T + b as 128-row
           tiles (PE-transposed x as lhsT, fp32r matmuls). Staged to DRAM
           in recurrence-friendly layouts:
             rzx_st [T, 64, 512]   bf16; row 16g+b = batch b, h-chunk g;
                                   free = [rx chunk | -zx chunk]
             nx_st  [T, 128, 256]  f32;  row 32g+b, free = nx chunk
             sk_st  [16, T, 1024]  bf16  (dense, per-batch)
  Phase 2: recurrence, all matmuls bf16. Gate pre-activations via 4-way
           PE column tiling: stationary = hT ko-tile [128, 16] shared by
           4 col-groups (tile_position (0, 32g)); each group streams its
           own 256-wide slice of the fused z-negated hidden weights
           (bf16 dodges the fp32r dst-partition ISA restriction). The
           PSUM layout packs the 4 h-chunks at partition bases
           0/32/64/96 so gate math runs as [128, 256] DVE/ACT ops.
           rx/zx/bhn adds fold into PSUM via a scatter-matmul
           (S: [64->128] row-scatter) that also initializes the
           accumulation groups. h_new is re-transposed with 2 full-array
           PE transposes (the (g,b) partition packing makes each
           [128,128] window transpose yield 4 hT ko-tiles at once).
  Phase 3: skip-add + LayerNorm + output projection (gamma/beta folded
           into Wout/bout on the host).
"""

import sys

for _p in ("/opt/trn_rl_repo", "/root/.axon_site/_ro/trn_rl_repo"):
    if _p not in sys.path:
        sys.path.insert(0, _p)

import numpy as np

import concourse.bass as bass
import concourse.tile as tile
from concourse import bacc, mybir
from concourse.bass_utils import run_bass_kernel_spmd

F32 = mybir.dt.float32
F32R = mybir.dt.float32r
BF16 = mybir.dt.bfloat16
AF = mybir.ActivationFunctionType
ALU = mybir.AluOpType

P = 128
B, T, I, H, O = 128, 1024, 512, 1024, 512
NCORES = 8
BC = B // NCORES  # 16 batch rows per core
NG = 4  # psum column groups (array col-tiles at bases 0/32/64/96)
HC = H // NG  # 256: h-chunk width per group
LN_EPS = 1e-5


def build_nc(t_steps: int = T):
    nc = bacc.Bacc(None, target_bir_lowering=False)

    # ---- I/O ----
    x_in = nc.dram_tensor("x", [BC, t_steps, I], F32, kind="ExternalInput")
    wiT = nc.dram_tensor("wiT", [I, 4 * H], F32R, kind="ExternalInput")
    bias_i = nc.dram_tensor("bias_i", [P, 4 * H], F32R, kind="ExternalInput")
    whT = nc.dram_tensor("whT", [H, 3 * H], BF16, kind="ExternalInput")
    bn_d = nc.dram_tensor("bn_d", [NG * BC, HC], BF16, kind="ExternalInput")
    scat = nc.dram_tensor("scat", [NG * BC, P], BF16, kind="ExternalInput")
    woT = nc.dram_tensor("woT", [H, O], F32R, kind="ExternalInput")
    bias_o = nc.dram_tensor("bias_o", [P, O], F32R, kind="ExternalInput")
    ones128 = nc.dram_tensor("ones128", [P, P], F32R, kind="ExternalInput")
    ident = nc.dram_tensor("ident", [P, P], F32, kind="ExternalInput")
    out = nc.dram_tensor("out", [BC, t_steps, O], F32, kind="ExternalOutput")

    n_rt = (BC * t_steps) // P  # number of 128-row tiles
    tpb = t_steps // P  # row-tiles ("time blocks") per batch row

    with tile.TileContext(nc) as tc:
        with (
            tc.tile_pool(name="dram", bufs=1, space="DRAM") as dram,
            tc.tile_pool(name="const", bufs=1) as const,
        ):
            # DRAM staging
            rzx_st = dram.tile([t_steps, NG * BC, 2 * HC], BF16)
            nx_st = dram.tile([t_steps, P, HC], F32)
            hs_st = dram.tile([t_steps, P, HC], BF16)
            sk_st = dram.tile([BC, t_steps, H], BF16)

            ident_sb = const.tile([P, P], F32)
            nc.sync.dma_start(ident_sb, ident[:])

            # ================= Phase 1: input projections =================
            with (
                tc.tile_pool(name="p1w", bufs=1) as p1w,
                tc.tile_pool(name="p1s", bufs=3) as p1s,
                tc.tile_pool(name="p1e", bufs=3) as p1e,
                tc.tile_pool(name="psA", bufs=2, space="PSUM") as psA,
                tc.tile_pool(name="psB", bufs=4, space="PSUM") as psB,
            ):
                wiT_sb = p1w.tile([P, I // P, 4 * H], F32R)
                nc.sync.dma_start(
                    wiT_sb, wiT[:].rearrange("(ko p) m -> p ko m", p=P)
                )
                bias_i_sb = p1w.tile([P, 4 * H], F32R)
                nc.sync.dma_start(bias_i_sb, bias_i[:])
                ones128_sb = p1w.tile([P, P], F32R)
                nc.sync.dma_start(ones128_sb, ones128[:])

                for rt in range(n_rt):
                    b = rt // tpb
                    t0 = (rt % tpb) * P
                    xt = p1s.tile([P, I], F32)
                    nc.sync.dma_start(xt, x_in[b, t0 : t0 + P, :])
                    px = psA.tile([P, I // P, P], F32)
                    for j in range(I // P):
                        nc.tensor.transpose(
                            px[:, j], xt[:, j * P : (j + 1) * P], ident_sb
                        )
                    xT = p1s.tile([P, I // P, P], F32R, tag="xT")
                    nc.vector.tensor_copy(xT, px)
                    for m in range(4):
                        if m < 2:
                            ev = p1e.tile([P, NG, HC], BF16, tag=f"ev{m}")
                        elif m == 2:
                            ev = p1e.tile([P, NG, HC], F32, tag="ev2")
                        else:
                            ev = p1e.tile([P, H], BF16, tag="ev3")
                        for c in range(2):
                            col = m * H + c * 512
                            pm = psB.tile([P, 512], F32)
                            for ko in range(I // P):
                                nc.tensor.matmul(
                                    pm,
                                    xT[:, ko],
                                    wiT_sb[:, ko, col : col + 512],
                                    start=(ko == 0),
                                    stop=False,
                                )
                            nc.tensor.matmul(
                                pm,
                                ones128_sb,
                                bias_i_sb[:, col : col + 512],
                                start=False,
                                stop=True,
                            )
                            if m < 3:
                                dstv = ev[:, 2 * c : 2 * c + 2, :]
                            else:
                                dstv = ev[:, c * 512 : (c + 1) * 512]
                            if c == 0:
                                nc.vector.tensor_copy(dstv, pm)
                            else:
                                nc.scalar.copy(dstv, pm)
                        if m == 0:  # r -> rzx_st free 0:HC
                            nc.sync.dma_start(
                                rzx_st[t0 : t0 + P, b::BC, 0:HC], ev
                            )
                        elif m == 1:  # -z -> rzx_st free HC:2HC
                            nc.sync.dma_start(
                                rzx_st[t0 : t0 + P, b::BC, HC : 2 * HC], ev
                            )
                        elif m == 2:  # n -> nx_st rows 32g+b
                            nc.sync.dma_start(
                                nx_st[t0 : t0 + P, b :: 2 * BC, :], ev
                            )
                        else:  # skip (bf16, dense)
                            nc.sync.dma_start(sk_st[b, t0 : t0 + P, :], ev)

            # ================= Phase 2: recurrence =================
            with (
                tc.tile_pool(name="p2w", bufs=1) as p2w,
                tc.tile_pool(name="p2s", bufs=4) as p2s,
                tc.tile_pool(name="p2t", bufs=2) as p2t,
                tc.tile_pool(name="pgr", bufs=1, space="PSUM") as pgr,
                tc.tile_pool(name="pgz", bufs=1, space="PSUM") as pgz,
                tc.tile_pool(name="pgn", bufs=1, space="PSUM") as pgn,
                tc.tile_pool(name="ptA", bufs=1, space="PSUM") as ptA,
                tc.tile_pool(name="ptB", bufs=1, space="PSUM") as ptB,
            ):
                whT_sb = p2w.tile([P, H // P, 3 * H], BF16)
                nc.sync.dma_start(
                    whT_sb, whT[:].rearrange("(ko p) m -> p ko m", p=P)
                )
                scat_sb = p2w.tile([NG * BC, P], BF16)
                nc.sync.dma_start(scat_sb, scat[:])
                bn_sb = p2w.tile([NG * BC, HC], BF16)
                nc.sync.dma_start(bn_sb, bn_d[:])

                # initial state h=0
                h_prev = p2t.tile([P, HC], F32, tag="h")
                nc.vector.memset(h_prev, 0.0)
                hT_prev = p2t.tile([P, 2, P], BF16, tag="hT")
                nc.vector.memset(hT_prev, 0.0)

                J_ORDER = [0, 2, 4, 6, 1, 3, 5, 7]  # w-major: trA consumers first

                for t in range(t_steps):
                    rzx_t = p2s.tile([NG * BC, 2 * HC], BF16, tag="rzx")
                    nc.sync.dma_start(rzx_t, rzx_st[t])
                    nx_t = p2s.tile([P, HC], F32, tag="nx")
                    nc.sync.dma_start(nx_t, nx_st[t])

                    ps_r = pgr.tile([P, HC], F32, tag="psr")
                    ps_z = pgz.tile([P, HC], F32, tag="psz")
                    ps_n = pgn.tile([P, HC], F32, tag="psn")
                    # scatter-inits: fold rx / (-zx) / bhn into PSUM
                    nc.tensor.matmul(
                        ps_r, scat_sb, rzx_t[:, 0:HC], start=True, stop=False
                    )
                    nc.tensor.matmul(ps_n, scat_sb, bn_sb, start=True, stop=False)
                    nc.tensor.matmul(
                        ps_z, scat_sb, rzx_t[:, HC : 2 * HC], start=True, stop=False
                    )

                    # gate matmuls: r (0), then n (2), then z (1)
                    for gate, ps in ((0, ps_r), (2, ps_n), (1, ps_z)):
                        for jx, j in enumerate(J_ORDER):
                            lh = hT_prev[
                                :, j % 2, 32 * (j // 2) : 32 * (j // 2) + BC
                            ]
                            for g in range(NG):
                                col = gate * H + HC * g
                                nc.tensor.matmul(
                                    ps[32 * g : 32 * g + BC, :],
                                    lh,
                                    whT_sb[:, j, col : col + HC],
                                    start=False,
                                    stop=(jx == 7 and g == NG - 1),
                                    tile_position=(0, 32 * g),
                                )
                        if gate == 0:
                            r_sb = p2t.tile([P, HC], F32, tag="r")
                            nc.scalar.activation(r_sb, ps_r, AF.Sigmoid)
                        elif gate == 2:
                            t1 = p2t.tile([P, HC], F32, tag="t1")
                            nc.vector.tensor_mul(t1, r_sb, ps_n)
                            t2 = p2t.tile([P, HC], F32, tag="t2")
                            nc.vector.tensor_add(t2, t1, nx_t)
                            n_sb = p2t.tile([P, HC], F32, tag="n")
                            nc.scalar.activation(n_sb, t2, AF.Tanh)
                            d_sb = p2t.tile([P, HC], F32, tag="d")
                            nc.vector.tensor_sub(d_sb, n_sb, h_prev)

                    zp_sb = p2t.tile([P, HC], F32, tag="zp")
                    nc.scalar.activation(zp_sb, ps_z, AF.Sigmoid)
                    e_sb = p2t.tile([P, HC], F32, tag="e")
                    nc.vector.tensor_mul(e_sb, zp_sb, d_sb)
                    h_new = p2t.tile([P, HC], F32, tag="h")
                    nc.vector.tensor_add(h_new, h_prev, e_sb)

                    # re-transpose h for next step's stationary operand
                    trA = ptA.tile([P, P], F32, tag="trA")
                    nc.tensor.transpose(trA, h_new[:, 0:P], ident_sb)
                    trB = ptB.tile([P, P], F32, tag="trB")
                    nc.tensor.transpose(trB, h_new[:, P : 2 * P], ident_sb)
                    hT_new = p2t.tile([P, 2, P], BF16, tag="hT")
                    nc.scalar.copy(hT_new[:, 0], trA)
                    nc.scalar.copy(hT_new[:, 1], trB)

                    h16 = p2s.tile([P, HC], BF16, tag="h16")
                    nc.scalar.copy(h16, h_new)
                    nc.sync.dma_start(hs_st[t], h16)

                    h_prev, hT_prev = h_new, hT_new

            # ================= Phase 3: skip + LN + out proj =================
            with (
                tc.tile_pool(name="p3w", bufs=1) as p3w,
                tc.tile_pool(name="p3s", bufs=3) as p3s,
                tc.tile_pool(name="p3t", bufs=2) as p3t,
                tc.tile_pool(name="ps3", bufs=2, space="PSUM") as ps3,
                tc.tile_pool(name="ps4", bufs=2, space="PSUM") as ps4,
            ):
                woT_sb = p3w.tile([P, H // P, O], F32R)
                nc.sync.dma_start(woT_sb, woT[:].rearrange("(ko p) m -> p ko m", p=P))
                bias_o_sb = p3w.tile([P, O], F32R)
                nc.sync.dma_start(bias_o_sb, bias_o[:])
                ones128_sb3 = p3w.tile([P, P], F32R)
                nc.sync.dma_start(ones128_sb3, ones128[:])
                eps_sb = p3w.tile([P, 1], F32)
                nc.vector.memset(eps_sb, LN_EPS)

                for rt in range(n_rt):
                    b = rt // tpb
                    t0 = (rt % tpb) * P
                    hs_t = p3s.tile([P, NG, HC], BF16, tag="hs")
                    nc.sync.dma_start(hs_t, hs_st[t0 : t0 + P, b :: 2 * BC, :])
                    sk_t = p3s.tile([P, H], BF16, tag="sk")
                    nc.sync.dma_start(sk_t, sk_st[b, t0 : t0 + P, :])
                    comb = p3t.tile([P, H], F32, tag="comb")
                    nc.vector.tensor_add(
                        comb, hs_t[:].rearrange("p g c -> p (g c)"), sk_t
                    )

                    st = p3t.tile([P, 2, 6], F32, tag="st")
                    nc.vector.bn_stats(st[:, 0], comb[:, :512])
                    nc.vector.bn_stats(st[:, 1], comb[:, 512:])
                    mv = p3t.tile([P, 2], F32, tag="mv")
                    nc.vector.bn_aggr(mv, st)
                    rstd = p3t.tile([P, 1], F32, tag="rstd")
                    nc.scalar.activation(rstd, mv[:, 1:2], AF.Sqrt, bias=eps_sb)
                    nc.vector.reciprocal(rstd, rstd)
                    normed = p3t.tile([P, H], F32, tag="normed")
                    nc.vector.tensor_scalar(
                        out=normed,
                        in0=comb,
                        scalar1=mv[:, 0:1],
                        scalar2=rstd,
                        op0=ALU.subtract,
                        op1=ALU.mult,
                    )

                    nT = p3t.tile([P, H // P, P], F32R, tag="nT")
                    for j2 in range(2):
                        pn = ps3.tile([P, 4, P], F32, tag="pn")
                        for j in range(4):
                            jj = j2 * 4 + j
                            nc.tensor.transpose(
                                pn[:, j], normed[:, jj * P : (jj + 1) * P], ident_sb
                            )
                        nc.vector.tensor_copy(nT[:, j2 * 4 : j2 * 4 + 4], pn)

                    po = ps4.tile([P, O], F32, tag="po")
                    for ko in range(H // P):
                        nc.tensor.matmul(
                            po, nT[:, ko], woT_sb[:, ko], start=(ko == 0), stop=False
                        )
                    nc.tensor.matmul(
                        po, ones128_sb3, bias_o_sb, start=False, stop=True
                    )
                    o_sb = p3t.tile([P, O], F32, tag="o")
                    nc.scalar.copy(o_sb, po)
                    nc.sync.dma_start(out[b, t0 : t0 + P, :], o_sb)

    nc.finalize()
    return nc


def prep_host_inputs(inputs):
    """Build the shared (weight) input arrays from the full problem inputs."""
    g = {k: np.asarray(v, dtype=np.float32) for k, v in inputs.items()}
    import ml_dtypes

    bf = ml_dtypes.bfloat16
    # z-path negated so one sigmoid yields zp = 1 - z directly
    wiT = np.concatenate(
        [g["Wir"].T, -g["Wiz"].T, g["Win"].T, g["Wskip"].T], axis=1
    )  # [I, 4H]
    bias_i = np.zeros((P, 4 * H), np.float32)
    bias_i[0, 0:H] = g["bir"] + g["bhr"]
    bias_i[0, H : 2 * H] = -(g["biz"] + g["bhz"])
    bias_i[0, 2 * H : 3 * H] = g["bin_"]
    bias_i[0, 3 * H :] = g["bskip"]
    whT = np.concatenate([g["Whr"].T, -g["Whz"].T, g["Whn"].T], axis=1)  # [H, 3H]
    bn_d = np.broadcast_to(
        g["bhn"].reshape(NG, 1, HC), (NG, BC, HC)
    ).reshape(NG * BC, HC).copy()
    scat = np.zeros((NG * BC, P), np.float32)
    for gg in range(NG):
        for bb in range(BC):
            scat[BC * gg + bb, 32 * gg + bb] = 1.0
    woT = np.ascontiguousarray((g["Wout"] * g["gamma"][None, :]).T)  # [H, O]
    bias_o = np.zeros((P, O), np.float32)
    bias_o[0] = g["bout"] + g["Wout"] @ g["beta"]
    ones128 = np.zeros((P, P), np.float32)
    ones128[0] = 1.0
    ident = np.eye(P, dtype=np.float32)
    return dict(
        wiT=np.ascontiguousarray(wiT),
        bias_i=bias_i,
        whT=np.ascontiguousarray(whT).astype(bf),
        bn_d=bn_d.astype(bf),
        scat=scat.astype(bf),
        woT=woT,
        bias_o=bias_o,
        ones128=ones128,
        ident=ident,
    )


_NC_CACHE = {}


def run(inputs, t_steps=T, trace=False):
    if t_steps not in _NC_CACHE:
        _NC_CACHE[t_steps] = build_nc(t_steps)
    nc = _NC_CACHE[t_steps]
    shared = prep_host_inputs(inputs)
    x = np.asarray(inputs["x"], dtype=np.float32)[:, :t_steps, :]
    in_maps = [
        {"x": np.ascontiguousarray(x[c * BC : (c + 1) * BC]), **shared}
        for c in range(NCORES)
    ]
    res = run_bass_kernel_spmd(
        nc, in_maps, core_ids=list(range(NCORES)), trace=trace
    )
    outp = np.concatenate([res.results[c]["out"] for c in range(NCORES)], axis=0)
    return outp, res


def kernel(**inputs) -> np.ndarray:
    outp, _ = run(inputs)
    return outp


# revision 7
# speedup vs baseline: 1.6726x; 1.1588x over previous
"""GRU-with-skip Trainium2 kernel (v2: 4-way column-tiled recurrence, bf16).

Strategy (data-parallel over batch, 8 cores, B_local=16 per core):
  Phase 1: input projections rx/(-zx)/nx/skip = x @ W*.T + b as 128-row
           tiles (PE-transposed x as lhsT, fp32r matmuls). Staged to DRAM
           in recurrence-friendly layouts:
             rzx_st [T, 64, 512]   bf16; row 16g+b = batch b, h-chunk g;
                                   free = [rx chunk | -zx chunk]
             nx_st  [T, 128, 256]  f32;  row 32g+b, free = nx chunk
             sk_st  [16, T, 1024]  bf16  (dense, per-batch)
  Phase 2: recurrence, all matmuls bf16. Gate pre-activations via 4-way
           PE column tiling: stationary = hT ko-tile [128, 16] shared by
           4 col-groups (tile_position (0, 32g)); each group streams its
           own 256-wide slice of the fused z-negated hidden weights
           (bf16 dodges the fp32r dst-partition ISA restriction). The
           PSUM layout packs the 4 h-chunks at partition bases
           0/32/64/96 so gate math runs as [128, 256] DVE/ACT ops.
           rx/zx/bhn adds fold into PSUM via a scatter-matmul
           (S: [64->128] row-scatter) that also initializes the
           accumulation groups. h_new is re-transposed with 2 full-array
           PE transposes (the (g,b) partition packing makes each
           [128,128] window transpose yield 4 hT ko-tiles at once).
  Phase 3: skip-add + LayerNorm + output projection (gamma/beta folded
           into Wout/bout on the host).
"""

import sys

for _p in ("/opt/trn_rl_repo", "/root/.axon_site/_ro/trn_rl_repo"):
    if _p not in sys.path:
        sys.path.insert(0, _p)

import numpy as np

import concourse.bass as bass
import concourse.tile as tile
from concourse import bacc, mybir
from concourse.bass_utils import run_bass_kernel_spmd

F32 = mybir.dt.float32
F32R = mybir.dt.float32r
BF16 = mybir.dt.bfloat16
AF = mybir.ActivationFunctionType
ALU = mybir.AluOpType

P = 128
B, T, I, H, O = 128, 1024, 512, 1024, 512
NCORES = 8
BC = B // NCORES  # 16 batch rows per core
NG = 4  # psum column groups (array col-tiles at bases 0/32/64/96)
HC = H // NG  # 256: h-chunk width per group
LN_EPS = 1e-5


def build_nc(t_steps: int = T):
    nc = bacc.Bacc(None, target_bir_lowering=False)

    # ---- I/O ----
    x_in = nc.dram_tensor("x", [BC, t_steps, I], F32, kind="ExternalInput")
    wiT = nc.dram_tensor("wiT", [I, 4 * H], F32R, kind="ExternalInput")
    bias_i = nc.dram_tensor("bias_i", [P, 4 * H], F32R, kind="ExternalInput")
    whT = nc.dram_tensor("whT", [H, 3 * H], BF16, kind="ExternalInput")
    bn_d = nc.dram_tensor("bn_d", [NG * BC, HC], BF16, kind="ExternalInput")
    scat = nc.dram_tensor("scat", [NG * BC, P], BF16, kind="ExternalInput")
    woT = nc.dram_tensor("woT", [H, O], F32R, kind="ExternalInput")
    bias_o = nc.dram_tensor("bias_o", [P, O], F32R, kind="ExternalInput")
    ones128 = nc.dram_tensor("ones128", [P, P], F32R, kind="ExternalInput")
    ident = nc.dram_tensor("ident", [P, P], F32, kind="ExternalInput")
    out = nc.dram_tensor("out", [BC, t_steps, O], F32, kind="ExternalOutput")

    n_rt = (BC * t_steps) // P  # number of 128-row tiles
    tpb = t_steps // P  # row-tiles ("time blocks") per batch row

    with tile.TileContext(nc) as tc:
        with (
            tc.tile_pool(name="dram", bufs=1, space="DRAM") as dram,
            tc.tile_pool(name="const", bufs=1) as const,
        ):
            # DRAM staging
            rzx_st = dram.tile([t_steps, NG * BC, 2 * HC], BF16)
            nx_st = dram.tile([t_steps, P, HC], F32)
            hs_st = dram.tile([t_steps, P, HC], BF16)
            sk_st = dram.tile([BC, t_steps, H], BF16)

            ident_sb = const.tile([P, P], F32)
            nc.sync.dma_start(ident_sb, ident[:])

            # ================= Phase 1: input projections =================
            with (
                tc.tile_pool(name="p1w", bufs=1) as p1w,
                tc.tile_pool(name="p1s", bufs=3) as p1s,
                tc.tile_pool(name="p1e", bufs=3) as p1e,
                tc.tile_pool(name="psA", bufs=2, space="PSUM") as psA,
                tc.tile_pool(name="psB", bufs=4, space="PSUM") as psB,
            ):
                wiT_sb = p1w.tile([P, I // P, 4 * H], F32R)
                nc.sync.dma_start(
                    wiT_sb, wiT[:].rearrange("(ko p) m -> p ko m", p=P)
                )
                bias_i_sb = p1w.tile([P, 4 * H], F32R)
                nc.sync.dma_start(bias_i_sb, bias_i[:])
                ones128_sb = p1w.tile([P, P], F32R)
                nc.sync.dma_start(ones128_sb, ones128[:])

                for rt in range(n_rt):
                    b = rt // tpb
                    t0 = (rt % tpb) * P
                    xt = p1s.tile([P, I], F32)
                    nc.sync.dma_start(xt, x_in[b, t0 : t0 + P, :])
                    px = psA.tile([P, I // P, P], F32)
                    for j in range(I // P):
                        nc.tensor.transpose(
                            px[:, j], xt[:, j * P : (j + 1) * P], ident_sb
                        )
                    xT = p1s.tile([P, I // P, P], F32R, tag="xT")
                    nc.vector.tensor_copy(xT, px)
                    for m in range(4):
                        if m < 2:
                            ev = p1e.tile([P, NG, HC], BF16, tag=f"ev{m}")
                        elif m == 2:
                            ev = p1e.tile([P, NG, HC], F32, tag="ev2")
                        else:
                            ev = p1e.tile([P, H], BF16, tag="ev3")
                        for c in range(2):
                            col = m * H + c * 512
                            pm = psB.tile([P, 512], F32)
                            for ko in range(I // P):
                                nc.tensor.matmul(
                                    pm,
                                    xT[:, ko],
                                    wiT_sb[:, ko, col : col + 512],
                                    start=(ko == 0),
                                    stop=False,
                                )
                            nc.tensor.matmul(
                                pm,
                                ones128_sb,
                                bias_i_sb[:, col : col + 512],
                                start=False,
                                stop=True,
                            )
                            if m < 3:
                                dstv = ev[:, 2 * c : 2 * c + 2, :]
                            else:
                                dstv = ev[:, c * 512 : (c + 1) * 512]
                            if c == 0:
                                nc.vector.tensor_copy(dstv, pm)
                            else:
                                nc.scalar.copy(dstv, pm)
                        if m == 0:  # r -> rzx_st free 0:HC
                            nc.sync.dma_start(
                                rzx_st[t0 : t0 + P, b::BC, 0:HC], ev
                            )
                        elif m == 1:  # -z -> rzx_st free HC:2HC
                            nc.sync.dma_start(
                                rzx_st[t0 : t0 + P, b::BC, HC : 2 * HC], ev
                            )
                        elif m == 2:  # n -> nx_st rows 32g+b
                            nc.sync.dma_start(
                                nx_st[t0 : t0 + P, b :: 2 * BC, :], ev
                            )
                        else:  # skip (bf16, dense)
                            nc.sync.dma_start(sk_st[b, t0 : t0 + P, :], ev)

            # ================= Phase 2: recurrence =================
            with (
                tc.tile_pool(name="p2w", bufs=1) as p2w,
                tc.tile_pool(name="p2s", bufs=4) as p2s,
                tc.tile_pool(name="p2t", bufs=2) as p2t,
                tc.tile_pool(name="pgr", bufs=1, space="PSUM") as pgr,
                tc.tile_pool(name="pgz", bufs=1, space="PSUM") as pgz,
                tc.tile_pool(name="pgn", bufs=1, space="PSUM") as pgn,
                tc.tile_pool(name="ptA", bufs=1, space="PSUM") as ptA,
                tc.tile_pool(name="ptB", bufs=1, space="PSUM") as ptB,
            ):
                whT_sb = p2w.tile([P, H // P, 3 * H], BF16)
                nc.sync.dma_start(
                    whT_sb, whT[:].rearrange("(ko p) m -> p ko m", p=P)
                )
                scat_sb = p2w.tile([NG * BC, P], BF16)
                nc.sync.dma_start(scat_sb, scat[:])
                bn_sb = p2w.tile([NG * BC, HC], BF16)
                nc.sync.dma_start(bn_sb, bn_d[:])

                # initial state h=0
                h_prev = p2t.tile([P, HC], F32, tag="h")
                nc.vector.memset(h_prev, 0.0)
                hT_prev = p2t.tile([P, 2, P], BF16, tag="hT")
                nc.vector.memset(hT_prev, 0.0)

                J_ORDER = [0, 2, 4, 6, 1, 3, 5, 7]  # w-major: trA consumers first

                for t in range(t_steps):
                    rzx_t = p2s.tile([NG * BC, 2 * HC], BF16, tag="rzx")
                    nc.sync.dma_start(rzx_t, rzx_st[t])
                    nx_t = p2s.tile([P, HC], F32, tag="nx")
                    nc.sync.dma_start(nx_t, nx_st[t])

                    ps_r = pgr.tile([P, HC], F32, tag="psr")
                    ps_z = pgz.tile([P, HC], F32, tag="psz")
                    ps_n = pgn.tile([P, HC], F32, tag="psn")
                    # scatter-inits: fold rx / (-zx) / bhn into PSUM
                    nc.tensor.matmul(
                        ps_r, scat_sb, rzx_t[:, 0:HC], start=True, stop=False
                    )
                    nc.tensor.matmul(ps_n, scat_sb, bn_sb, start=True, stop=False)
                    nc.tensor.matmul(
                        ps_z, scat_sb, rzx_t[:, HC : 2 * HC], start=True, stop=False
                    )

                    # gate matmuls: r (0), then n (2), then z (1)
                    import os as _os

                    group_major = bool(int(_os.environ.get("GROUP_MAJOR", "0")))
                    for gate, ps in ((0, ps_r), (2, ps_n), (1, ps_z)):
                        if group_major:
                            order = [
                                (jx, j, g)
                                for g in range(NG)
                                for jx, j in enumerate(J_ORDER)
                            ]
                        else:
                            order = [
                                (jx, j, g)
                                for jx, j in enumerate(J_ORDER)
                                for g in range(NG)
                            ]
                        for jx, j, g in order:
                            lh = hT_prev[
                                :, j % 2, 32 * (j // 2) : 32 * (j // 2) + BC
                            ]
                            col = gate * H + HC * g
                            nc.tensor.matmul(
                                ps[32 * g : 32 * g + BC, :],
                                lh,
                                whT_sb[:, j, col : col + HC],
                                start=False,
                                stop=(jx == 7 and g == NG - 1),
                                tile_position=(0, 32 * g),
                            )
                        if gate == 0:
                            r_sb = p2t.tile([P, HC], F32, tag="r")
                            nc.scalar.activation(r_sb, ps_r, AF.Sigmoid)
                        elif gate == 2:
                            t1 = p2t.tile([P, HC], F32, tag="t1")
                            nc.vector.tensor_mul(t1, r_sb, ps_n)
                            t2 = p2t.tile([P, HC], F32, tag="t2")
                            nc.vector.tensor_add(t2, t1, nx_t)
                            n_sb = p2t.tile([P, HC], F32, tag="n")
                            nc.scalar.activation(n_sb, t2, AF.Tanh)
                            d_sb = p2t.tile([P, HC], F32, tag="d")
                            nc.vector.tensor_sub(d_sb, n_sb, h_prev)

                    zp_sb = p2t.tile([P, HC], F32, tag="zp")
                    nc.scalar.activation(zp_sb, ps_z, AF.Sigmoid)
                    e_sb = p2t.tile([P, HC], F32, tag="e")
                    nc.vector.tensor_mul(e_sb, zp_sb, d_sb)
                    h_new = p2t.tile([P, HC], F32, tag="h")
                    nc.vector.tensor_add(h_new, h_prev, e_sb)

                    # re-transpose h for next step's stationary operand
                    trA = ptA.tile([P, P], F32, tag="trA")
                    nc.tensor.transpose(trA, h_new[:, 0:P], ident_sb)
                    trB = ptB.tile([P, P], F32, tag="trB")
                    nc.tensor.transpose(trB, h_new[:, P : 2 * P], ident_sb)
                    hT_new = p2t.tile([P, 2, P], BF16, tag="hT")
                    nc.scalar.copy(hT_new[:, 0], trA)
                    nc.scalar.copy(hT_new[:, 1], trB)

                    h16 = p2s.tile([P, HC], BF16, tag="h16")
                    nc.scalar.copy(h16, h_new)
                    nc.sync.dma_start(hs_st[t], h16)

                    h_prev, hT_prev = h_new, hT_new

            # ================= Phase 3: skip + LN + out proj =================
            with (
                tc.tile_pool(name="p3w", bufs=1) as p3w,
                tc.tile_pool(name="p3s", bufs=3) as p3s,
                tc.tile_pool(name="p3t", bufs=2) as p3t,
                tc.tile_pool(name="ps3", bufs=2, space="PSUM") as ps3,
                tc.tile_pool(name="ps4", bufs=2, space="PSUM") as ps4,
            ):
                woT_sb = p3w.tile([P, H // P, O], F32R)
                nc.sync.dma_start(woT_sb, woT[:].rearrange("(ko p) m -> p ko m", p=P))
                bias_o_sb = p3w.tile([P, O], F32R)
                nc.sync.dma_start(bias_o_sb, bias_o[:])
                ones128_sb3 = p3w.tile([P, P], F32R)
                nc.sync.dma_start(ones128_sb3, ones128[:])
                eps_sb = p3w.tile([P, 1], F32)
                nc.vector.memset(eps_sb, LN_EPS)

                for rt in range(n_rt):
                    b = rt // tpb
                    t0 = (rt % tpb) * P
                    hs_t = p3s.tile([P, NG, HC], BF16, tag="hs")
                    nc.sync.dma_start(hs_t, hs_st[t0 : t0 + P, b :: 2 * BC, :])
                    sk_t = p3s.tile([P, H], BF16, tag="sk")
                    nc.sync.dma_start(sk_t, sk_st[b, t0 : t0 + P, :])
                    comb = p3t.tile([P, H], F32, tag="comb")
                    nc.vector.tensor_add(
                        comb, hs_t[:].rearrange("p g c -> p (g c)"), sk_t
                    )

                    st = p3t.tile([P, 2, 6], F32, tag="st")
                    nc.vector.bn_stats(st[:, 0], comb[:, :512])
                    nc.vector.bn_stats(st[:, 1], comb[:, 512:])
                    mv = p3t.tile([P, 2], F32, tag="mv")
                    nc.vector.bn_aggr(mv, st)
                    rstd = p3t.tile([P, 1], F32, tag="rstd")
                    nc.scalar.activation(rstd, mv[:, 1:2], AF.Sqrt, bias=eps_sb)
                    nc.vector.reciprocal(rstd, rstd)
                    normed = p3t.tile([P, H], F32, tag="normed")
                    nc.vector.tensor_scalar(
                        out=normed,
                        in0=comb,
                        scalar1=mv[:, 0:1],
                        scalar2=rstd,
                        op0=ALU.subtract,
                        op1=ALU.mult,
                    )

                    nT = p3t.tile([P, H // P, P], F32R, tag="nT")
                    for j2 in range(2):
                        pn = ps3.tile([P, 4, P], F32, tag="pn")
                        for j in range(4):
                            jj = j2 * 4 + j
                            nc.tensor.transpose(
                                pn[:, j], normed[:, jj * P : (jj + 1) * P], ident_sb
                            )
                        nc.vector.tensor_copy(nT[:, j2 * 4 : j2 * 4 + 4], pn)

                    po = ps4.tile([P, O], F32, tag="po")
                    for ko in range(H // P):
                        nc.tensor.matmul(
                            po, nT[:, ko], woT_sb[:, ko], start=(ko == 0), stop=False
                        )
                    nc.tensor.matmul(
                        po, ones128_sb3, bias_o_sb, start=False, stop=True
                    )
                    o_sb = p3t.tile([P, O], F32, tag="o")
                    nc.scalar.copy(o_sb, po)
                    nc.sync.dma_start(out[b, t0 : t0 + P, :], o_sb)

    nc.finalize()
    return nc


def prep_host_inputs(inputs):
    """Build the shared (weight) input arrays from the full problem inputs."""
    g = {k: np.asarray(v, dtype=np.float32) for k, v in inputs.items()}
    import ml_dtypes

    bf = ml_dtypes.bfloat16
    # z-path negated so one sigmoid yields zp = 1 - z directly
    wiT = np.concatenate(
        [g["Wir"].T, -g["Wiz"].T, g["Win"].T, g["Wskip"].T], axis=1
    )  # [I, 4H]
    bias_i = np.zeros((P, 4 * H), np.float32)
    bias_i[0, 0:H] = g["bir"] + g["bhr"]
    bias_i[0, H : 2 * H] = -(g["biz"] + g["bhz"])
    bias_i[0, 2 * H : 3 * H] = g["bin_"]
    bias_i[0, 3 * H :] = g["bskip"]
    whT = np.concatenate([g["Whr"].T, -g["Whz"].T, g["Whn"].T], axis=1)  # [H, 3H]
    bn_d = np.broadcast_to(
        g["bhn"].reshape(NG, 1, HC), (NG, BC, HC)
    ).reshape(NG * BC, HC).copy()
    scat = np.zeros((NG * BC, P), np.float32)
    for gg in range(NG):
        for bb in range(BC):
            scat[BC * gg + bb, 32 * gg + bb] = 1.0
    woT = np.ascontiguousarray((g["Wout"] * g["gamma"][None, :]).T)  # [H, O]
    bias_o = np.zeros((P, O), np.float32)
    bias_o[0] = g["bout"] + g["Wout"] @ g["beta"]
    ones128 = np.zeros((P, P), np.float32)
    ones128[0] = 1.0
    ident = np.eye(P, dtype=np.float32)
    return dict(
        wiT=np.ascontiguousarray(wiT),
        bias_i=bias_i,
        whT=np.ascontiguousarray(whT).astype(bf),
        bn_d=bn_d.astype(bf),
        scat=scat.astype(bf),
        woT=woT,
        bias_o=bias_o,
        ones128=ones128,
        ident=ident,
    )


_NC_CACHE = {}


def run(inputs, t_steps=T, trace=False):
    if t_steps not in _NC_CACHE:
        _NC_CACHE[t_steps] = build_nc(t_steps)
    nc = _NC_CACHE[t_steps]
    shared = prep_host_inputs(inputs)
    x = np.asarray(inputs["x"], dtype=np.float32)[:, :t_steps, :]
    in_maps = [
        {"x": np.ascontiguousarray(x[c * BC : (c + 1) * BC]), **shared}
        for c in range(NCORES)
    ]
    res = run_bass_kernel_spmd(
        nc, in_maps, core_ids=list(range(NCORES)), trace=trace
    )
    outp = np.concatenate([res.results[c]["out"] for c in range(NCORES)], axis=0)
    return outp, res


def kernel(**inputs) -> np.ndarray:
    outp, _ = run(inputs)
    return outp
